# revision 8
# baseline (speedup 1.0000x reference)
"""Trainium2 Bass kernel for the VQ commitment-loss problem.

Math
----
reference loss = 0.25 * mean((codebook[argmin_k dist] - flat)**2)
               = 0.25/(B*T*D) * sum_n min_k ||flat_n - e_k||^2
since the gathered quantized row realizes exactly the min squared distance.

min_k ||f - e||^2 = ||f||^2 + min_k (||e_k||^2 - 2 f.e_k)

So per core (2 of 16 batches):
  - sum_n ||f_n||^2 via the window-count trick:
        sum over tau of cnt(tau) * x_pad[tau]^2   (cnt = #windows containing tau)
  - the min term via bf16 TensorE matmuls of window tiles against the
    codebook (scaled by -2), with ||e_k||^2 folded into the contraction as
    two extra rows (hi/lo bf16 split of the norm, paired with ones in lhsT),
    then a VectorE min-reduce over the 1024 codes straight out of PSUM.

Host side only pads/casts/shards inputs and sums the 8 per-core partial
scalars.
"""

import numpy as np
import ml_dtypes

B, P, T = 16, 12, 4096
WIN = 41
PAD = (WIN - 1) // 2          # 20
K = 1024
D = P * WIN                   # 492
COMMITMENT_COST = 0.25

NCORES = 8
BC = B // NCORES              # batches per core = 2
TP = T + 2 * PAD              # padded time = 4136
NCHUNK = 4                    # contraction chunks: 3 pellets * 41 taps = 123 rows
CHROWS = 3 * WIN              # 123
WBLK = 512                    # windows per DMA block
NBLK = BC * T // WBLK         # 16
MSUB = WBLK // 128            # 4 matmul subtiles per block
NSUB = NBLK * MSUB            # 64 subtiles per core
TCHUNK = TP // 4              # 1034 (xsq layout)

SCALE = COMMITMENT_COST / (B * T * D)

_CACHED = {}


def _build_nc():
    import concourse.bacc as bacc
    import concourse.bass as bass
    import concourse.mybir as mybir
    import concourse.tile as tile

    BF = mybir.dt.bfloat16
    F32 = mybir.dt.float32
    AX = mybir.AxisListType
    OP = mybir.AluOpType

    nc = bacc.Bacc("TRN2", target_bir_lowering=False, debug=False)

    xw_d = nc.dram_tensor("xw", [BC, P, TP], BF, kind="ExternalInput")
    cb_d = nc.dram_tensor("cb", [NCHUNK, 128, K], BF, kind="ExternalInput")
    cnt_d = nc.dram_tensor("cnt", [96, TCHUNK], F32, kind="ExternalInput")
    out_d = nc.dram_tensor("out", [1, 1], F32, kind="ExternalOutput")

    with tile.TileContext(nc) as tc:
        with (
            tc.tile_pool(name="cbpool", bufs=1) as cbpool,
            tc.tile_pool(name="wpool", bufs=2) as wpool,
            tc.tile_pool(name="misc", bufs=1) as misc,
        ):
            # ---- resident codebook tiles (rows = contraction dims, free = codes)
            cbt = []
            for c in range(NCHUNK):
                cbtile = cbpool.tile([128, K], BF, tag=f"cb{c}", name=f"cbt{c}")
                nc.sync.dma_start(cbtile[:], cb_d[c])
                cbt.append(cbtile)

            ones_bf = misc.tile([CHROWS, 1], BF)
            nc.vector.memset(ones_bf[:], 1.0)
            ones2 = misc.tile([2, WBLK], BF)
            nc.vector.memset(ones2[:], 1.0)
            ones_f = misc.tile([128, 1], F32)
            nc.vector.memset(ones_f[:], 1.0)
            mins_a = misc.tile([128, NSUB], F32)
            mins_b = misc.tile([128, NSUB], F32)

            # ---- prologue: c_k = ||e_k||^2 as hi/lo bf16 rows into cbt[0][123:125]
            with (
                tc.tile_pool(name="pre", bufs=1) as pre,
                tc.tile_pool(name="ppre", bufs=1, space="PSUM") as ppre,
            ):
                sqt = []
                for c in range(NCHUNK):
                    sq = pre.tile([CHROWS, K], BF, tag=f"sq{c}", name=f"sqt{c}")
                    # (-2e)^2 = 4 e^2 ; scaled back by 0.25 below
                    nc.scalar.square(sq[:], cbt[c][0:CHROWS, :])
                    sqt.append(sq)
                pcs = []
                for h in range(2):
                    pc = ppre.tile([1, 512], F32, tag=f"pc{h}", name=f"pc{h}")
                    for c in range(NCHUNK):
                        nc.tensor.matmul(
                            pc[:],
                            ones_bf[:],
                            sqt[c][:, 512 * h : 512 * (h + 1)],
                            start=(c == 0),
                            stop=(c == NCHUNK - 1),
                        )
                    pcs.append(pc)
                cq = pre.tile([1, K], F32)
                for h in range(2):
                    nc.vector.tensor_scalar_mul(
                        cq[:, 512 * h : 512 * (h + 1)], pcs[h][:], 0.25
                    )
                chi = pre.tile([1, K], BF)
                nc.vector.tensor_copy(chi[:], cq[:])
                chif = pre.tile([1, K], F32)
                nc.vector.tensor_copy(chif[:], chi[:])
                clof = pre.tile([1, K], F32)
                nc.vector.tensor_sub(clof[:], cq[:], chif[:])
                clo = pre.tile([1, K], BF)
                nc.vector.tensor_copy(clo[:], clof[:])
                nc.sync.dma_start(cbt[0][CHROWS : CHROWS + 1, :], chi[:])
                nc.sync.dma_start(cbt[0][CHROWS + 1 : CHROWS + 2, :], clo[:])

            # ---- sum_n ||f_n||^2 term: sum cnt(tau) * x^2
            xsq_in = misc.tile([96, TCHUNK], BF)
            nc.sync.dma_start(
                xsq_in[:],
                bass.AP(
                    xw_d,
                    0,
                    [[P * TP, BC], [TP, P], [TCHUNK, 4], [1, TCHUNK]],
                ),
            )
            cnt_sb = misc.tile([96, TCHUNK], F32)
            nc.sync.dma_start(cnt_sb[:], cnt_d[:])
            sqx = misc.tile([96, TCHUNK], F32)
            nc.vector.tensor_mul(sqx[:], xsq_in[:], xsq_in[:])
            wsq = misc.tile([96, TCHUNK], F32)
            nc.vector.tensor_mul(wsq[:], sqx[:], cnt_sb[:])
            selfsum = misc.tile([96, 1], F32)
            nc.vector.tensor_reduce(selfsum[:], wsq[:], axis=AX.X, op=OP.add)

            # ---- main loop: 16 blocks x 4 subtiles of 128 windows
            with tc.tile_pool(name="pmain", bufs=3, space="PSUM") as pmain:
                for blk in range(NBLK):
                    b, t0 = blk // (NBLK // BC), (blk % (NBLK // BC)) * WBLK
                    wt = []
                    for c in range(NCHUNK):
                        kc = CHROWS + 2 if c == 0 else CHROWS
                        w = wpool.tile([kc, WBLK], BF, tag=f"w{c}", name=f"w{c}_{blk}")
                        nc.sync.dma_start(
                            w[0:CHROWS, :],
                            bass.AP(
                                xw_d,
                                (b * P + 3 * c) * TP + t0,
                                [[TP, 3], [1, WIN], [1, WBLK]],
                            ),
                        )
                        if c == 0:
                            nc.sync.dma_start(w[CHROWS : CHROWS + 2, :], ones2[:])
                        wt.append(w)
                    for m in range(MSUB):
                        i = blk * MSUB + m
                        for h, mbuf in ((0, mins_a), (1, mins_b)):
                            ps = pmain.tile(
                                [128, 512], F32, tag=f"ps{h}", name=f"ps_{blk}_{m}_{h}"
                            )
                            for c in range(NCHUNK):
                                kc = CHROWS + 2 if c == 0 else CHROWS
                                nc.tensor.matmul(
                                    ps[:],
                                    wt[c][0:kc, 128 * m : 128 * (m + 1)],
                                    cbt[c][0:kc, 512 * h : 512 * (h + 1)],
                                    start=(c == 0),
                                    stop=(c == NCHUNK - 1),
                                )
                            nc.vector.tensor_reduce(
                                mbuf[:, i : i + 1], ps[:], axis=AX.X, op=OP.min
                            )

            # ---- finale: grand sum -> scale -> out
            minp = misc.tile([128, NSUB], F32)
            nc.vector.tensor_tensor(minp[:], mins_a[:], mins_b[:], op=OP.min)
            macc = misc.tile([128, 1], F32)
            nc.vector.tensor_reduce(macc[:], minp[:], axis=AX.X, op=OP.add)
            with tc.tile_pool(name="pfin", bufs=1, space="PSUM") as pfin:
                fin = pfin.tile([1, 1], F32)
                nc.tensor.matmul(fin[:], macc[:], ones_f[:], start=True, stop=False)
                nc.tensor.matmul(
                    fin[:], selfsum[:], ones_f[0:96, :], start=False, stop=True
                )
                res = misc.tile([1, 1], F32)
                nc.vector.tensor_scalar_mul(res[:], fin[:], float(SCALE))
                nc.sync.dma_start(out_d[:], res[:])

    nc.compile()
    return nc


def get_nc():
    if "nc" not in _CACHED:
        _CACHED["nc"] = _build_nc()
    return _CACHED["nc"]


def _host_prep(x, codebook):
    """Pad/cast/shard the inputs; returns per-core in_maps."""
    x = np.asarray(x, dtype=np.float32)
    codebook = np.asarray(codebook, dtype=np.float32)

    xb = x.astype(ml_dtypes.bfloat16)
    xw = np.zeros((B, P, TP), dtype=ml_dtypes.bfloat16)
    xw[:, :, PAD : PAD + T] = xb

    # value of the bf16-rounded codebook, exactly scaled by -2
    cbb = codebook.astype(ml_dtypes.bfloat16).astype(np.float32)
    rhs = np.zeros((NCHUNK, 128, K), dtype=np.float32)
    for c in range(NCHUNK):
        rhs[c, :CHROWS, :] = (-2.0 * cbb[:, CHROWS * c : CHROWS * (c + 1)]).T
    rhs_bf = rhs.astype(ml_dtypes.bfloat16)

    tau = np.arange(TP, dtype=np.float32)
    cnt = np.minimum(np.minimum(tau + 1.0, float(WIN)), float(TP) - tau)
    cnt_rep = np.tile(cnt.reshape(4, TCHUNK), (BC * P, 1)).astype(np.float32)

    in_maps = []
    for i in range(NCORES):
        in_maps.append(
            {
                "xw": np.ascontiguousarray(xw[BC * i : BC * (i + 1)]),
                "cb": rhs_bf,
                "cnt": cnt_rep,
            }
        )
    return in_maps


def kernel(x, codebook):
    from concourse.bass_utils import run_bass_kernel_spmd

    nc = get_nc()
    in_maps = _host_prep(x, codebook)
    res = run_bass_kernel_spmd(nc, in_maps, core_ids=list(range(NCORES)))
    total = np.float64(0.0)
    for r in res.results:
        total += np.float64(r["out"][0, 0])
    return np.array(np.float32(total))
